# revision 18
# baseline (speedup 1.0000x reference)
"""AAGNN GraphConvolution kernel for 8 Trainium2 NeuronCores.

Computes relu(degree_norm * (adj @ (x @ W)) + b) for
x[16384,128], adj[16384,16384], degree_norm[16384,1], W[128,64], b[64].

Sharding: 1D row partition of the output nodes across 8 cores (2048 rows
each). Each core receives the transposed row-block of the adjacency
(adjT[16384, 2048], contiguous) so the TensorEngine can contract over the
full node axis with contiguous DMA, plus the (host-precomputed) support
x @ W and its degree_norm slice. No cross-core communication is needed.

Adjacency compression: entries are uniform [0,1), so the stream is
mean-centered (adj - 0.5) and quantized to fp8 E3M4 on host. For |v| <= 0.5
the E3M4 grid is uniform (step 2^-6, subnormals included), which halves the
DMA bytes versus bf16 at ~1e-2 relative error. The removed mean is a rank-1
term 0.5 * ones @ support, restored by pre-loading each PSUM accumulator
with c[h] = 0.5 * colsum(x @ W) (host-computed) via a K=1 matmul c x ones
before the adjacency stream starts (start=False on the first stream matmul).

Device program per core (fp8 adjacency stream, bf16 support, fp32 PSUM):
  aggT = c*ones + support_kb-stationary matmuls over the adjT stream, two
         PE column halves computing two m-slices concurrently -> PSUM
  out  = relu(deg * aggT + b)  in a scrambled [128, 1024] layout
Support chunks ride the same HWDGE ring interleaved with their adjacency
tile, so tile t's operands arrive together and the PE starts ~6us in.
Host packs adjT into DMA tiles and unscrambles the outputs to [16384, 64].
"""

import sys
import types

if "/opt/trn_rl_repo" not in sys.path:
    sys.path.insert(0, "/opt/trn_rl_repo")

import numpy as np
import ml_dtypes

import concourse.bass as bass  # noqa: F401  (AP helpers)
import concourse.mybir as mybir
import concourse.tile as tile
from concourse import bacc
from concourse.bass_utils import run_bass_kernel_spmd


def _ensure_ntff_hook():
    """bass_utils imports antenv.axon_hooks when tracing is requested
    (trace=True or BASS_TRACE=1). This image's antenv lacks that module, so
    rebuild the hook from trn_agent_boot's ctypes shim — or register a None
    hook so tracing degrades gracefully instead of raising ImportError."""
    try:
        import antenv.axon_hooks  # noqa: F401

        return
    except ImportError:
        pass
    hook = None
    try:
        from trn_agent_boot.trn_boot import _ntff_profile_via_ctypes

        hook = _ntff_profile_via_ctypes("/opt/axon/libaxon_pjrt.so")
    except Exception:
        hook = None
    mod = types.ModuleType("antenv.axon_hooks")
    mod.get_axon_ntff_profile_hook = lambda: hook
    mod.set_axon_ntff_profile_hook = lambda h: None
    sys.modules["antenv.axon_hooks"] = mod


_ensure_ntff_hook()

N_NODES = 16384
F = 128  # feature size
H = 64  # hidden size
N_CORES = 8
ROWS = N_NODES // N_CORES  # 2048 output rows per core
KB = 128  # contraction block (partition dim)

# Tunables
ADJ_MODE = "fp8"  # adjacency stream dtype: "fp8" (E3M4, centered) | "bf16" | "f32"
ADJ_BUFS = 40  # in-flight adjacency DMA tiles
KB_PER_TILE = 2  # k-blocks per adjacency DMA tile
SUP_CHUNKS = 8  # support DMA chunks interleaved into the SP stream

_ADJ_DT = {
    "fp8": (mybir.dt.float8e3, ml_dtypes.float8_e3m4),
    "bf16": (mybir.dt.bfloat16, ml_dtypes.bfloat16),
    "f32": (mybir.dt.float32, np.float32),
}


def build_nc(
    n_nodes: int = N_NODES,
    rows: int = ROWS,
    adj_mode: str = ADJ_MODE,
    adj_bufs: int = ADJ_BUFS,
    kb_per_tile: int = KB_PER_TILE,
    sup_chunks: int = SUP_CHUNKS,
):
    """Build the single-core Bass program (same program on every core)."""
    f32 = mybir.dt.float32
    qdt = _ADJ_DT[adj_mode][0]  # adjacency stream dtype
    sdt = f32 if adj_mode == "f32" else mybir.dt.bfloat16  # support dtype
    nkb = n_nodes // KB  # number of contraction blocks

    # Column-pairing: two concurrent matmuls on PE column halves compute two
    # different m-slices of the output. Output/deg live in a scrambled
    # [128, rows/2] layout: partition p, col i*n_slice+n  <->
    # (h = p%64, m = i*2*n_slice + (p//64)*n_slice + n); host unscrambles.
    n_slice = min(512, rows // 2)
    n_pairs = rows // (2 * n_slice)
    hcols = n_pairs * n_slice  # rows // 2

    n_tiles = nkb // kb_per_tile
    nc = bacc.Bacc("TRN2", debug=False, num_devices=N_CORES)
    # adjacency arrives host-pre-tiled: row t*128+p holds the kb_per_tile
    # k-block chunks of partition p for tile t, so each DMA tile is one
    # fully-contiguous DRAM block with 16KB-contiguous per-partition runs
    adjT = nc.declare_dram_parameter(
        "adjT", [n_tiles * KB, kb_per_tile * rows], qdt, isOutput=False
    )
    # support = x @ W, host-precomputed, [k partition, kb*H free] layout
    supp = nc.declare_dram_parameter("sup", [KB, nkb * H], sdt, isOutput=False)
    bp = nc.declare_dram_parameter("b", [2 * H, 1], f32, isOutput=False)
    # all small epilogue constants ride one fp16 DMA: deg (the two distinct
    # rows of the scrambled layout), a partition-half selector (a K=2 matmul
    # broadcasts deg to all 128 partitions on-device), the rank-1
    # mean-restore row cr[p] = 0.5 * colsum(x @ W)[p % 64], and a ones row
    fp16 = mybir.dt.float16
    c_deg, c_sel, c_cr, c_ones = 0, hcols, hcols + 2 * H, hcols + 4 * H
    cn = hcols + 4 * H + n_slice
    constp = nc.declare_dram_parameter("consts", [2, cn], fp16, isOutput=False)
    outp = nc.declare_dram_parameter("out", [2 * H, hcols], f32, isOutput=True)

    with tile.TileContext(nc) as tc:
        with (
            tc.tile_pool(name="const", bufs=1) as cpool,
            tc.tile_pool(name="adj", bufs=adj_bufs) as apool,
            tc.tile_pool(name="spsum", bufs=2, space="PSUM") as spool,
            tc.tile_pool(name="accs", bufs=1, space="PSUM") as accpool,
            tc.tile_pool(name="epi", bufs=2) as epool,
        ):
            # ---- constants + first support chunk at the head of the SP
            # ring (the ACT ring is serialized behind its activation-table
            # load; SP has nothing ahead, so the PE can seed PSUM and start
            # on tile 0 at ~8us). Remaining support chunks are interleaved
            # into the adjacency stream right where they are first needed.
            con_sb = cpool.tile([2, cn], fp16, tag="consts")
            nc.sync.dma_start(out=con_sb[:], in_=constp[:, :])
            support_sb = cpool.tile([KB, nkb * H], sdt, tag="support")
            sup_dma = nkb * H // sup_chunks
            nc.sync.dma_start(out=support_sb[:, :sup_dma], in_=supp[:, :sup_dma])
            b_sb = cpool.tile([2 * H, 1], f32, tag="b")
            nc.scalar.dma_start(out=b_sb[:], in_=bp[:, :])

            # ---- deg broadcast + accumulator pre-init, before the stream ----
            # (PE is otherwise idle while the first adjacency tile arrives)
            deg_sb = cpool.tile([2 * H, hcols], f32, tag="deg")
            for i in range(n_pairs):
                dps = spool.tile([2 * H, n_slice], f32, tag="spsum", name="dps")
                nc.tensor.matmul(
                    out=dps[:],
                    lhsT=con_sb[:, c_sel : c_sel + 2 * H],
                    rhs=con_sb[:, c_deg + i * n_slice : c_deg + (i + 1) * n_slice],
                    start=True,
                    stop=True,
                )
                nc.vector.tensor_copy(
                    out=deg_sb[:, i * n_slice : (i + 1) * n_slice], in_=dps[:]
                )
            accs = [
                accpool.tile([2 * H, n_slice], f32, tag=f"acc{i}", name=f"acc{i}")
                for i in range(n_pairs)
            ]
            for i in range(n_pairs):
                # rank-1 mean-restore term c x ones seeds the accumulation
                nc.tensor.matmul(
                    out=accs[i][:, :],
                    lhsT=con_sb[0:1, c_cr : c_cr + 2 * H],
                    rhs=con_sb[0:1, c_ones : c_ones + n_slice],
                    start=True,
                    stop=False,
                    skip_group_check=True,
                )

            # ---- aggregation: aggT[h, m] += support_kb.T-stationary @ adjT ----
            # Every k-block issues 2*n_pairs accumulating matmuls; within a
            # pair the two matmuls target different PE column halves
            # (tile_position) and run concurrently on two m-slices.
            def agg_mm(a, t, j, i, u):
                kb = t * kb_per_tile + j
                m0 = (2 * i + u) * n_slice
                nc.tensor.matmul(
                    out=accs[i][u * H : (u + 1) * H, :],
                    lhsT=support_sb[:, kb * H : (kb + 1) * H],
                    rhs=a[:, j * rows + m0 : j * rows + m0 + n_slice],
                    start=False,
                    stop=(kb == nkb - 1),
                    tile_position=(0, u * H),
                    # the two column halves are disjoint partition groups in
                    # the same bank; the coarse zero-region group check can't
                    # express that
                    skip_group_check=True,
                )

            tiles_per_sup = n_tiles // sup_chunks
            # chunk g enters the FIFO ~6 tiles before its first consumer
            sup_issue = {max(1, g * tiles_per_sup - 6): g for g in range(1, sup_chunks)}
            for t in range(n_tiles):
                if t in sup_issue:
                    g = sup_issue[t]
                    nc.sync.dma_start(
                        out=support_sb[:, g * sup_dma : (g + 1) * sup_dma],
                        in_=supp[:, g * sup_dma : (g + 1) * sup_dma],
                    )
                a = apool.tile([KB, kb_per_tile * rows], qdt, tag="adj", name="a")
                nc.sync.dma_start(out=a[:], in_=adjT[t * KB : (t + 1) * KB, :])
                if t < n_tiles - 1:
                    for j in range(kb_per_tile):
                        for i in range(n_pairs):
                            for u in (0, 1):
                                agg_mm(a, t, j, i, u)
                else:
                    # last tile: finish pair 0 first so its epilogue overlaps
                    # pair 1's final matmuls
                    for i in range(n_pairs):
                        for j in range(kb_per_tile):
                            for u in (0, 1):
                                agg_mm(a, t, j, i, u)

            # ---- epilogue: relu(deg * aggT + b), in the scrambled layout ----
            o_sb = epool.tile([2 * H, hcols], f32, tag="o", name="o")
            for i in range(n_pairs):
                tmp = epool.tile([2 * H, n_slice], f32, tag="tmp", name="tmp")
                nc.vector.tensor_tensor(
                    out=tmp[:],
                    in0=accs[i][:],
                    in1=deg_sb[:, i * n_slice : (i + 1) * n_slice],
                    op=mybir.AluOpType.mult,
                )
                nc.scalar.activation(
                    out=o_sb[:, i * n_slice : (i + 1) * n_slice],
                    in_=tmp[:],
                    func=mybir.ActivationFunctionType.Relu,
                    bias=b_sb[:],
                )
                # per-pair output DMA overlaps the other pair's epilogue
                nc.scalar.dma_start(
                    out=outp[:, i * n_slice : (i + 1) * n_slice],
                    in_=o_sb[:, i * n_slice : (i + 1) * n_slice],
                )

    nc.compile()
    return nc


def pack_adjT(adjT_c, rows, kb_per_tile=KB_PER_TILE):
    """[n_nodes, rows] transposed adjacency shard -> DMA-tiled layout
    [n_tiles*128, kb_per_tile*rows]: row t*128+p concatenates the
    kb_per_tile k-block rows (kb_per_tile*t+j)*128+p, giving contiguous
    per-partition runs inside each DMA tile."""
    n_nodes = adjT_c.shape[0]
    n_tiles = n_nodes // (KB * kb_per_tile)
    return np.ascontiguousarray(
        adjT_c.reshape(n_tiles, kb_per_tile, KB, rows)
        .transpose(0, 2, 1, 3)
        .reshape(n_tiles * KB, kb_per_tile * rows)
    )


def pack_support(sup, dtype):
    """[n_nodes, H] support -> [128, nkb*H]: partition k, col kb*H+h holds
    support[kb*128 + k, h]."""
    n_nodes = sup.shape[0]
    nkb = n_nodes // KB
    return np.ascontiguousarray(
        sup.reshape(nkb, KB, H).transpose(1, 0, 2).reshape(KB, nkb * H).astype(dtype)
    )


def scramble_cols(v, rows):
    """[rows] vector -> [2, rows//2]: the two distinct rows of the kernel's
    scrambled layout (row u, col i*ns+n = v[i*2*ns + u*ns + n]); the kernel
    broadcasts row u to partitions u*64..u*64+63 via a K=2 selector matmul."""
    ns = min(512, rows // 2)
    npair = rows // (2 * ns)
    m = v.reshape(npair, 2, ns)  # [i, u, n]
    out = np.empty((2, npair * ns), dtype=v.dtype)
    for u in (0, 1):
        out[u, :] = m[:, u, :].reshape(npair * ns)
    return out


SEL = np.zeros((2, 2 * H), dtype=np.float32)
SEL[0, :H] = 1.0
SEL[1, H:] = 1.0


def unscramble_out(o, rows):
    """[128, rows//2] kernel output -> [rows, H] natural layout."""
    ns = min(512, rows // 2)
    npair = rows // (2 * ns)
    outT = np.empty((H, rows), dtype=o.dtype)
    for i in range(npair):
        for u in (0, 1):
            outT[:, (2 * i + u) * ns : (2 * i + u + 1) * ns] = o[
                u * H : (u + 1) * H, i * ns : (i + 1) * ns
            ]
    return outT.T


def make_in_maps(x, adj_matrix, degree_norm, W, b, adj_mode=ADJ_MODE,
                 kb_per_tile=KB_PER_TILE):
    """Shard the full inputs into per-core input maps (host-side, numpy)."""
    qdt = _ADJ_DT[adj_mode][1]
    sdt = np.float32 if adj_mode == "f32" else ml_dtypes.bfloat16
    center = adj_mode == "fp8"
    n_nodes = x.shape[0]
    rows = n_nodes // N_CORES
    ns = min(512, rows // 2)
    # support precomputed on host in fp32, shipped in sdt
    sup_f32 = x.astype(np.float32) @ np.asarray(W, np.float32)
    supm = pack_support(sup_f32, sdt)
    bf = np.ascontiguousarray(
        np.concatenate([np.asarray(b, np.float32)] * 2), dtype=np.float32
    ).reshape(2 * H, 1)
    if center:
        # exact rank-1 mean-restore: c = 0.5 * colsum(x @ W)
        c = 0.5 * sup_f32.astype(np.float64).sum(axis=0).astype(np.float32)
    else:
        c = np.zeros(H, np.float32)
    in_maps = []
    for ci in range(N_CORES):
        r0, r1 = ci * rows, (ci + 1) * rows
        adjT_c = adj_matrix[r0:r1, :].T.astype(np.float32)
        if center:
            adjT_c = adjT_c - np.float32(0.5)
        adjT_c = pack_adjT(
            np.ascontiguousarray(adjT_c.astype(qdt)), rows, kb_per_tile=kb_per_tile
        )
        deg_c = scramble_cols(
            np.ascontiguousarray(degree_norm[r0:r1].reshape(-1), np.float32), rows
        )
        # packed fp16 constants: [deg2 | sel | cr | ones] (see build_nc)
        hcols = rows // 2
        consts = np.zeros((2, hcols + 4 * H + ns), np.float16)
        consts[:, :hcols] = deg_c
        consts[:, hcols : hcols + 2 * H] = SEL
        consts[0, hcols + 2 * H : hcols + 4 * H] = np.concatenate([c, c])
        consts[0, hcols + 4 * H :] = 1.0
        in_maps.append({"adjT": adjT_c, "sup": supm, "b": bf, "consts": consts})
    return in_maps


_nc_cache = {}


def _get_nc():
    key = (ADJ_MODE, ADJ_BUFS, KB_PER_TILE, SUP_CHUNKS)
    if key not in _nc_cache:
        _nc_cache[key] = build_nc()
    return _nc_cache[key]


def kernel(x, adj_matrix, degree_norm, W, b):
    x = np.asarray(x)
    adj_matrix = np.asarray(adj_matrix)
    degree_norm = np.asarray(degree_norm)
    W = np.asarray(W)
    b = np.asarray(b)

    nc = _get_nc()
    in_maps = make_in_maps(x, adj_matrix, degree_norm, W, b)
    try:
        res = run_bass_kernel_spmd(nc, in_maps, core_ids=list(range(N_CORES)))
    except Exception:
        # transient NRT_EXEC_UNIT_UNRECOVERABLE after an aborted prior run
        # heals after touching the devices once; retry a single time
        try:
            import jax, jax.numpy as jnp  # noqa: E401

            for d in jax.devices():
                jnp.add(jax.device_put(jnp.ones((2, 2)), d), 1.0).block_until_ready()
        except Exception:
            pass
        res = run_bass_kernel_spmd(nc, in_maps, core_ids=list(range(N_CORES)))
    out = np.empty((N_NODES, H), dtype=np.float32)
    for c in range(N_CORES):
        out[c * ROWS : (c + 1) * ROWS, :] = unscramble_out(res.results[c]["out"], ROWS)
    return out


# revision 21
# speedup vs baseline: 1.0764x; 1.0764x over previous
"""AAGNN GraphConvolution kernel for 8 Trainium2 NeuronCores.

Computes relu(degree_norm * (adj @ (x @ W)) + b) for
x[16384,128], adj[16384,16384], degree_norm[16384,1], W[128,64], b[64].

Sharding: 1D row partition of the output nodes across 8 cores (2048 rows
each). Each core receives the transposed row-block of the adjacency
(adjT[16384, 2048], contiguous) so the TensorEngine can contract over the
full node axis with contiguous DMA, plus the (host-precomputed) support
x @ W and its degree_norm slice. No cross-core communication is needed.

Adjacency compression: entries are uniform [0,1), so the stream is
mean-centered (adj - 0.5) and quantized to fp8 E3M4 on host. For |v| <= 0.5
the E3M4 grid is uniform (step 2^-6, subnormals included), which halves the
DMA bytes versus bf16 at ~1e-2 relative error. The removed mean is a rank-1
term 0.5 * ones @ support, restored by pre-loading each PSUM accumulator
with c[h] = 0.5 * colsum(x @ W) (host-computed) via a K=1 matmul c x ones
before the adjacency stream starts (start=False on the first stream matmul).

Device program per core (fp8 adjacency stream, bf16 support, fp32 PSUM):
  aggT = c*ones + support_kb-stationary matmuls over the adjT stream, two
         PE column halves computing two m-slices concurrently -> PSUM
  out  = relu(deg * aggT + b)  in a scrambled [128, 1024] layout
Support chunks ride the same HWDGE ring interleaved with their adjacency
tile, so tile t's operands arrive together and the PE starts ~6us in.
Host packs adjT into DMA tiles and unscrambles the outputs to [16384, 64].
"""

import sys
import types

if "/opt/trn_rl_repo" not in sys.path:
    sys.path.insert(0, "/opt/trn_rl_repo")

import numpy as np
import ml_dtypes

import concourse.bass as bass  # noqa: F401  (AP helpers)
import concourse.mybir as mybir
import concourse.tile as tile
from concourse import bacc
from concourse.bass_utils import run_bass_kernel_spmd


def _ensure_ntff_hook():
    """bass_utils imports antenv.axon_hooks when tracing is requested
    (trace=True or BASS_TRACE=1). This image's antenv lacks that module, so
    rebuild the hook from trn_agent_boot's ctypes shim — or register a None
    hook so tracing degrades gracefully instead of raising ImportError."""
    try:
        import antenv.axon_hooks  # noqa: F401

        return
    except ImportError:
        pass
    hook = None
    try:
        from trn_agent_boot.trn_boot import _ntff_profile_via_ctypes

        hook = _ntff_profile_via_ctypes("/opt/axon/libaxon_pjrt.so")
    except Exception:
        hook = None
    mod = types.ModuleType("antenv.axon_hooks")
    mod.get_axon_ntff_profile_hook = lambda: hook
    mod.set_axon_ntff_profile_hook = lambda h: None
    sys.modules["antenv.axon_hooks"] = mod


_ensure_ntff_hook()

N_NODES = 16384
F = 128  # feature size
H = 64  # hidden size
N_CORES = 8
ROWS = N_NODES // N_CORES  # 2048 output rows per core
KB = 128  # contraction block (partition dim)

# Tunables
ADJ_MODE = "fp8"  # adjacency stream dtype: "fp8" (E3M4, centered) | "bf16" | "f32"
ADJ_BUFS = 20  # in-flight adjacency DMA tiles
KB_PER_TILE = 4  # k-blocks per adjacency DMA tile
SUP_CHUNKS = 2  # support: small SP head chunk + bulk on ACT ring

_ADJ_DT = {
    "fp8": (mybir.dt.float8e3, ml_dtypes.float8_e3m4),
    "bf16": (mybir.dt.bfloat16, ml_dtypes.bfloat16),
    "f32": (mybir.dt.float32, np.float32),
}


def build_nc(
    n_nodes: int = N_NODES,
    rows: int = ROWS,
    adj_mode: str = ADJ_MODE,
    adj_bufs: int = ADJ_BUFS,
    kb_per_tile: int = KB_PER_TILE,
    sup_chunks: int = SUP_CHUNKS,
):
    """Build the single-core Bass program (same program on every core)."""
    f32 = mybir.dt.float32
    qdt = _ADJ_DT[adj_mode][0]  # adjacency stream dtype
    sdt = f32 if adj_mode == "f32" else mybir.dt.bfloat16  # support dtype
    nkb = n_nodes // KB  # number of contraction blocks

    # Column-pairing: two concurrent matmuls on PE column halves compute two
    # different m-slices of the output. Output/deg live in a scrambled
    # [128, rows/2] layout: partition p, col i*n_slice+n  <->
    # (h = p%64, m = i*2*n_slice + (p//64)*n_slice + n); host unscrambles.
    n_slice = min(512, rows // 2)
    n_pairs = rows // (2 * n_slice)
    hcols = n_pairs * n_slice  # rows // 2

    n_tiles = nkb // kb_per_tile
    nc = bacc.Bacc("TRN2", debug=False, num_devices=N_CORES)
    # adjacency arrives host-pre-tiled: row t*128+p holds the kb_per_tile
    # k-block chunks of partition p for tile t, so each DMA tile is one
    # fully-contiguous DRAM block with 16KB-contiguous per-partition runs
    adjT = nc.declare_dram_parameter(
        "adjT", [n_tiles * KB, kb_per_tile * rows], qdt, isOutput=False
    )
    # support = x @ W, host-precomputed, [k partition, kb*H free] layout
    supp = nc.declare_dram_parameter("sup", [KB, nkb * H], sdt, isOutput=False)
    bp = nc.declare_dram_parameter("b", [2 * H, 1], f32, isOutput=False)
    # all small epilogue constants ride one fp16 DMA: deg (the two distinct
    # rows of the scrambled layout), a partition-half selector (a K=2 matmul
    # broadcasts deg to all 128 partitions on-device), the rank-1
    # mean-restore row cr[p] = 0.5 * colsum(x @ W)[p % 64], and a ones row
    fp16 = mybir.dt.float16
    c_deg, c_sel, c_cr, c_ones = 0, hcols, hcols + 2 * H, hcols + 4 * H
    cn = hcols + 4 * H + n_slice
    constp = nc.declare_dram_parameter("consts", [2, cn], fp16, isOutput=False)
    outp = nc.declare_dram_parameter("out", [2 * H, hcols], f32, isOutput=True)

    with tile.TileContext(nc) as tc:
        with (
            tc.tile_pool(name="const", bufs=1) as cpool,
            tc.tile_pool(name="adj", bufs=adj_bufs) as apool,
            tc.tile_pool(name="spsum", bufs=2, space="PSUM") as spool,
            tc.tile_pool(name="accs", bufs=1, space="PSUM") as accpool,
            tc.tile_pool(name="epi", bufs=2) as epool,
        ):
            # ---- constants + a small support head-chunk on the SP ring
            # (the ACT ring is serialized behind its activation-table load;
            # SP has nothing ahead, so the PE can seed PSUM and start on
            # tile 0 at ~11us). The bulk of the support rides the ACT ring
            # and lands well before its first consumer tile.
            sup_head = 4 * kb_per_tile * H  # support cols for the first 4 tiles
            con_sb = cpool.tile([2, cn], fp16, tag="consts")
            nc.sync.dma_start(out=con_sb[:], in_=constp[:, :])
            support_sb = cpool.tile([KB, nkb * H], sdt, tag="support")
            nc.sync.dma_start(out=support_sb[:, :sup_head], in_=supp[:, :sup_head])
            nc.scalar.dma_start(
                out=support_sb[:, sup_head:], in_=supp[:, sup_head:]
            )
            b_sb = cpool.tile([2 * H, 1], f32, tag="b")
            nc.scalar.dma_start(out=b_sb[:], in_=bp[:, :])

            # ---- deg broadcast + accumulator pre-init, before the stream ----
            # (PE is otherwise idle while the first adjacency tile arrives)
            deg_sb = cpool.tile([2 * H, hcols], f32, tag="deg")
            for i in range(n_pairs):
                dps = spool.tile([2 * H, n_slice], f32, tag="spsum", name="dps")
                nc.tensor.matmul(
                    out=dps[:],
                    lhsT=con_sb[:, c_sel : c_sel + 2 * H],
                    rhs=con_sb[:, c_deg + i * n_slice : c_deg + (i + 1) * n_slice],
                    start=True,
                    stop=True,
                )
                nc.vector.tensor_copy(
                    out=deg_sb[:, i * n_slice : (i + 1) * n_slice], in_=dps[:]
                )
            accs = [
                accpool.tile([2 * H, n_slice], f32, tag=f"acc{i}", name=f"acc{i}")
                for i in range(n_pairs)
            ]
            for i in range(n_pairs):
                # rank-1 mean-restore term c x ones seeds the accumulation
                nc.tensor.matmul(
                    out=accs[i][:, :],
                    lhsT=con_sb[0:1, c_cr : c_cr + 2 * H],
                    rhs=con_sb[0:1, c_ones : c_ones + n_slice],
                    start=True,
                    stop=False,
                    skip_group_check=True,
                )

            # ---- aggregation: aggT[h, m] += support_kb.T-stationary @ adjT ----
            # Every k-block issues 2*n_pairs accumulating matmuls; within a
            # pair the two matmuls target different PE column halves
            # (tile_position) and run concurrently on two m-slices.
            def agg_mm(a, t, j, i, u):
                kb = t * kb_per_tile + j
                m0 = (2 * i + u) * n_slice
                nc.tensor.matmul(
                    out=accs[i][u * H : (u + 1) * H, :],
                    lhsT=support_sb[:, kb * H : (kb + 1) * H],
                    rhs=a[:, j * rows + m0 : j * rows + m0 + n_slice],
                    start=False,
                    stop=(kb == nkb - 1),
                    tile_position=(0, u * H),
                    # the two column halves are disjoint partition groups in
                    # the same bank; the coarse zero-region group check can't
                    # express that
                    skip_group_check=True,
                )

            for t in range(n_tiles):
                a = apool.tile([KB, kb_per_tile * rows], qdt, tag="adj", name="a")
                nc.sync.dma_start(out=a[:], in_=adjT[t * KB : (t + 1) * KB, :])
                if t < n_tiles - 1:
                    for j in range(kb_per_tile):
                        for i in range(n_pairs):
                            for u in (0, 1):
                                agg_mm(a, t, j, i, u)
                else:
                    # last tile: finish pair 0 first so its epilogue overlaps
                    # pair 1's final matmuls
                    for i in range(n_pairs):
                        for j in range(kb_per_tile):
                            for u in (0, 1):
                                agg_mm(a, t, j, i, u)

            # ---- epilogue: relu(deg * aggT + b), in the scrambled layout ----
            o_sb = epool.tile([2 * H, hcols], f32, tag="o", name="o")
            for i in range(n_pairs):
                tmp = epool.tile([2 * H, n_slice], f32, tag="tmp", name="tmp")
                nc.vector.tensor_tensor(
                    out=tmp[:],
                    in0=accs[i][:],
                    in1=deg_sb[:, i * n_slice : (i + 1) * n_slice],
                    op=mybir.AluOpType.mult,
                )
                nc.scalar.activation(
                    out=o_sb[:, i * n_slice : (i + 1) * n_slice],
                    in_=tmp[:],
                    func=mybir.ActivationFunctionType.Relu,
                    bias=b_sb[:],
                )
                # per-pair output DMA overlaps the other pair's epilogue
                nc.scalar.dma_start(
                    out=outp[:, i * n_slice : (i + 1) * n_slice],
                    in_=o_sb[:, i * n_slice : (i + 1) * n_slice],
                )

    nc.compile()
    return nc


def pack_adjT(adjT_c, rows, kb_per_tile=KB_PER_TILE):
    """[n_nodes, rows] transposed adjacency shard -> DMA-tiled layout
    [n_tiles*128, kb_per_tile*rows]: row t*128+p concatenates the
    kb_per_tile k-block rows (kb_per_tile*t+j)*128+p, giving contiguous
    per-partition runs inside each DMA tile."""
    n_nodes = adjT_c.shape[0]
    n_tiles = n_nodes // (KB * kb_per_tile)
    return np.ascontiguousarray(
        adjT_c.reshape(n_tiles, kb_per_tile, KB, rows)
        .transpose(0, 2, 1, 3)
        .reshape(n_tiles * KB, kb_per_tile * rows)
    )


def pack_support(sup, dtype):
    """[n_nodes, H] support -> [128, nkb*H]: partition k, col kb*H+h holds
    support[kb*128 + k, h]."""
    n_nodes = sup.shape[0]
    nkb = n_nodes // KB
    return np.ascontiguousarray(
        sup.reshape(nkb, KB, H).transpose(1, 0, 2).reshape(KB, nkb * H).astype(dtype)
    )


def scramble_cols(v, rows):
    """[rows] vector -> [2, rows//2]: the two distinct rows of the kernel's
    scrambled layout (row u, col i*ns+n = v[i*2*ns + u*ns + n]); the kernel
    broadcasts row u to partitions u*64..u*64+63 via a K=2 selector matmul."""
    ns = min(512, rows // 2)
    npair = rows // (2 * ns)
    m = v.reshape(npair, 2, ns)  # [i, u, n]
    out = np.empty((2, npair * ns), dtype=v.dtype)
    for u in (0, 1):
        out[u, :] = m[:, u, :].reshape(npair * ns)
    return out


SEL = np.zeros((2, 2 * H), dtype=np.float32)
SEL[0, :H] = 1.0
SEL[1, H:] = 1.0


def unscramble_out(o, rows):
    """[128, rows//2] kernel output -> [rows, H] natural layout."""
    ns = min(512, rows // 2)
    npair = rows // (2 * ns)
    outT = np.empty((H, rows), dtype=o.dtype)
    for i in range(npair):
        for u in (0, 1):
            outT[:, (2 * i + u) * ns : (2 * i + u + 1) * ns] = o[
                u * H : (u + 1) * H, i * ns : (i + 1) * ns
            ]
    return outT.T


def make_in_maps(x, adj_matrix, degree_norm, W, b, adj_mode=ADJ_MODE,
                 kb_per_tile=KB_PER_TILE):
    """Shard the full inputs into per-core input maps (host-side, numpy)."""
    qdt = _ADJ_DT[adj_mode][1]
    sdt = np.float32 if adj_mode == "f32" else ml_dtypes.bfloat16
    center = adj_mode == "fp8"
    n_nodes = x.shape[0]
    rows = n_nodes // N_CORES
    ns = min(512, rows // 2)
    # support precomputed on host in fp32, shipped in sdt
    sup_f32 = x.astype(np.float32) @ np.asarray(W, np.float32)
    supm = pack_support(sup_f32, sdt)
    bf = np.ascontiguousarray(
        np.concatenate([np.asarray(b, np.float32)] * 2), dtype=np.float32
    ).reshape(2 * H, 1)
    if center:
        # exact rank-1 mean-restore: c = 0.5 * colsum(x @ W)
        c = 0.5 * sup_f32.astype(np.float64).sum(axis=0).astype(np.float32)
    else:
        c = np.zeros(H, np.float32)
    in_maps = []
    for ci in range(N_CORES):
        r0, r1 = ci * rows, (ci + 1) * rows
        adjT_c = adj_matrix[r0:r1, :].T.astype(np.float32)
        if center:
            adjT_c = adjT_c - np.float32(0.5)
        adjT_c = pack_adjT(
            np.ascontiguousarray(adjT_c.astype(qdt)), rows, kb_per_tile=kb_per_tile
        )
        deg_c = scramble_cols(
            np.ascontiguousarray(degree_norm[r0:r1].reshape(-1), np.float32), rows
        )
        # packed fp16 constants: [deg2 | sel | cr | ones] (see build_nc)
        hcols = rows // 2
        consts = np.zeros((2, hcols + 4 * H + ns), np.float16)
        consts[:, :hcols] = deg_c
        consts[:, hcols : hcols + 2 * H] = SEL
        consts[0, hcols + 2 * H : hcols + 4 * H] = np.concatenate([c, c])
        consts[0, hcols + 4 * H :] = 1.0
        in_maps.append({"adjT": adjT_c, "sup": supm, "b": bf, "consts": consts})
    return in_maps


_nc_cache = {}


def _get_nc():
    key = (ADJ_MODE, ADJ_BUFS, KB_PER_TILE, SUP_CHUNKS)
    if key not in _nc_cache:
        _nc_cache[key] = build_nc()
    return _nc_cache[key]


def kernel(x, adj_matrix, degree_norm, W, b):
    x = np.asarray(x)
    adj_matrix = np.asarray(adj_matrix)
    degree_norm = np.asarray(degree_norm)
    W = np.asarray(W)
    b = np.asarray(b)

    nc = _get_nc()
    in_maps = make_in_maps(x, adj_matrix, degree_norm, W, b)
    try:
        res = run_bass_kernel_spmd(nc, in_maps, core_ids=list(range(N_CORES)))
    except Exception:
        # transient NRT_EXEC_UNIT_UNRECOVERABLE after an aborted prior run
        # heals after touching the devices once; retry a single time
        try:
            import jax, jax.numpy as jnp  # noqa: E401

            for d in jax.devices():
                jnp.add(jax.device_put(jnp.ones((2, 2)), d), 1.0).block_until_ready()
        except Exception:
            pass
        res = run_bass_kernel_spmd(nc, in_maps, core_ids=list(range(N_CORES)))
    out = np.empty((N_NODES, H), dtype=np.float32)
    for c in range(N_CORES):
        out[c * ROWS : (c + 1) * ROWS, :] = unscramble_out(res.results[c]["out"], ROWS)
    return out


# revision 22
# speedup vs baseline: 1.1114x; 1.0326x over previous
"""AAGNN GraphConvolution kernel for 8 Trainium2 NeuronCores.

Computes relu(degree_norm * (adj @ (x @ W)) + b) for
x[16384,128], adj[16384,16384], degree_norm[16384,1], W[128,64], b[64].

Sharding: 1D row partition of the output nodes across 8 cores (2048 rows
each). Each core receives the transposed row-block of the adjacency
(adjT[16384, 2048], contiguous) so the TensorEngine can contract over the
full node axis with contiguous DMA, plus the (host-precomputed) support
x @ W and its degree_norm slice. No cross-core communication is needed.

Adjacency compression: entries are uniform [0,1), so the stream is
mean-centered (adj - 0.5) and quantized to fp8 E3M4 on host. For |v| <= 0.5
the E3M4 grid is uniform (step 2^-6, subnormals included), which halves the
DMA bytes versus bf16 at ~1e-2 relative error. The removed mean is a rank-1
term 0.5 * ones @ support, restored by pre-loading each PSUM accumulator
with c[h] = 0.5 * colsum(x @ W) (host-computed) via a K=1 matmul c x ones
before the adjacency stream starts (start=False on the first stream matmul).

Device program per core (fp8 adjacency stream, bf16 support, fp32 PSUM):
  aggT = c*ones + support_kb-stationary matmuls over the adjT stream, two
         PE column halves computing two m-slices concurrently -> PSUM
  out  = relu(deg * aggT + b)  in a scrambled [128, 1024] layout
Support chunks ride the same HWDGE ring interleaved with their adjacency
tile, so tile t's operands arrive together and the PE starts ~6us in.
Host packs adjT into DMA tiles and unscrambles the outputs to [16384, 64].
"""

import sys
import types

if "/opt/trn_rl_repo" not in sys.path:
    sys.path.insert(0, "/opt/trn_rl_repo")

import numpy as np
import ml_dtypes

import concourse.bass as bass  # noqa: F401  (AP helpers)
import concourse.mybir as mybir
import concourse.tile as tile
from concourse import bacc
from concourse.bass_utils import run_bass_kernel_spmd


def _ensure_ntff_hook():
    """bass_utils imports antenv.axon_hooks when tracing is requested
    (trace=True or BASS_TRACE=1). This image's antenv lacks that module, so
    rebuild the hook from trn_agent_boot's ctypes shim — or register a None
    hook so tracing degrades gracefully instead of raising ImportError."""
    try:
        import antenv.axon_hooks  # noqa: F401

        return
    except ImportError:
        pass
    hook = None
    try:
        from trn_agent_boot.trn_boot import _ntff_profile_via_ctypes

        hook = _ntff_profile_via_ctypes("/opt/axon/libaxon_pjrt.so")
    except Exception:
        hook = None
    mod = types.ModuleType("antenv.axon_hooks")
    mod.get_axon_ntff_profile_hook = lambda: hook
    mod.set_axon_ntff_profile_hook = lambda h: None
    sys.modules["antenv.axon_hooks"] = mod


_ensure_ntff_hook()

N_NODES = 16384
F = 128  # feature size
H = 64  # hidden size
N_CORES = 8
ROWS = N_NODES // N_CORES  # 2048 output rows per core
KB = 128  # contraction block (partition dim)

# Tunables
ADJ_MODE = "fp8"  # adjacency stream dtype: "fp8" (E3M4, centered) | "bf16" | "f32"
ADJ_BUFS = 20  # in-flight adjacency DMA tiles
KB_PER_TILE = 4  # k-blocks per adjacency DMA tile
SUP_CHUNKS = 2  # support: small SP head chunk + bulk on ACT ring

_ADJ_DT = {
    "fp8": (mybir.dt.float8e3, ml_dtypes.float8_e3m4),
    "bf16": (mybir.dt.bfloat16, ml_dtypes.bfloat16),
    "f32": (mybir.dt.float32, np.float32),
}


def build_nc(
    n_nodes: int = N_NODES,
    rows: int = ROWS,
    adj_mode: str = ADJ_MODE,
    adj_bufs: int = ADJ_BUFS,
    kb_per_tile: int = KB_PER_TILE,
    sup_chunks: int = SUP_CHUNKS,
):
    """Build the single-core Bass program (same program on every core)."""
    f32 = mybir.dt.float32
    qdt = _ADJ_DT[adj_mode][0]  # adjacency stream dtype
    sdt = f32 if adj_mode == "f32" else mybir.dt.bfloat16  # support dtype
    nkb = n_nodes // KB  # number of contraction blocks

    # Column-pairing: two concurrent matmuls on PE column halves compute two
    # different m-slices of the output. Output/deg live in a scrambled
    # [128, rows/2] layout: partition p, col i*n_slice+n  <->
    # (h = p%64, m = i*2*n_slice + (p//64)*n_slice + n); host unscrambles.
    n_slice = min(512, rows // 2)
    n_pairs = rows // (2 * n_slice)
    hcols = n_pairs * n_slice  # rows // 2

    n_tiles = nkb // kb_per_tile
    nc = bacc.Bacc("TRN2", debug=False, num_devices=N_CORES)
    # adjacency arrives host-pre-tiled: row t*128+p holds the kb_per_tile
    # k-block chunks of partition p for tile t, so each DMA tile is one
    # fully-contiguous DRAM block with 16KB-contiguous per-partition runs
    adjT = nc.declare_dram_parameter(
        "adjT", [n_tiles * KB, kb_per_tile * rows], qdt, isOutput=False
    )
    # support = x @ W, host-precomputed, [k partition, kb*H free] layout
    supp = nc.declare_dram_parameter("sup", [KB, nkb * H], sdt, isOutput=False)
    bp = nc.declare_dram_parameter("b", [2 * H, 1], f32, isOutput=False)
    # all small epilogue constants ride one fp16 DMA: deg (the two distinct
    # rows of the scrambled layout), a partition-half selector (a K=2 matmul
    # broadcasts deg to all 128 partitions on-device), the rank-1
    # mean-restore row cr[p] = 0.5 * colsum(x @ W)[p % 64], and a ones row
    fp16 = mybir.dt.float16
    c_deg, c_sel, c_cr, c_ones = 0, hcols, hcols + 2 * H, hcols + 4 * H
    cn = hcols + 4 * H + n_slice
    constp = nc.declare_dram_parameter("consts", [2, cn], fp16, isOutput=False)
    outp = nc.declare_dram_parameter("out", [2 * H, hcols], f32, isOutput=True)

    with tile.TileContext(nc) as tc:
        with (
            tc.tile_pool(name="const", bufs=1) as cpool,
            tc.tile_pool(name="adj", bufs=adj_bufs) as apool,
            tc.tile_pool(name="spsum", bufs=2, space="PSUM") as spool,
            tc.tile_pool(name="accs", bufs=1, space="PSUM") as accpool,
            tc.tile_pool(name="epi", bufs=2) as epool,
        ):
            # ---- constants + a small support head-chunk on the SP ring
            # (the ACT ring is serialized behind its activation-table load;
            # SP has nothing ahead, so the PE can seed PSUM and start on
            # tile 0 at ~11us). The bulk of the support rides the ACT ring
            # and lands well before its first consumer tile.
            sup_head = min(4 * kb_per_tile, nkb) * H  # first 4 tiles' support
            con_sb = cpool.tile([2, cn], fp16, tag="consts")
            nc.sync.dma_start(out=con_sb[:], in_=constp[:, :])
            support_sb = cpool.tile([KB, nkb * H], sdt, tag="support")
            nc.sync.dma_start(out=support_sb[:, :sup_head], in_=supp[:, :sup_head])
            if sup_head < nkb * H:
                nc.scalar.dma_start(
                    out=support_sb[:, sup_head:], in_=supp[:, sup_head:]
                )
            b_sb = cpool.tile([2 * H, 1], f32, tag="b")
            nc.scalar.dma_start(out=b_sb[:], in_=bp[:, :])

            # ---- deg broadcast + accumulator pre-init, before the stream ----
            # (PE is otherwise idle while the first adjacency tile arrives)
            deg_sb = cpool.tile([2 * H, hcols], f32, tag="deg")
            for i in range(n_pairs):
                dps = spool.tile([2 * H, n_slice], f32, tag="spsum", name="dps")
                nc.tensor.matmul(
                    out=dps[:],
                    lhsT=con_sb[:, c_sel : c_sel + 2 * H],
                    rhs=con_sb[:, c_deg + i * n_slice : c_deg + (i + 1) * n_slice],
                    start=True,
                    stop=True,
                )
                nc.vector.tensor_copy(
                    out=deg_sb[:, i * n_slice : (i + 1) * n_slice], in_=dps[:]
                )
            accs = [
                accpool.tile([2 * H, n_slice], f32, tag=f"acc{i}", name=f"acc{i}")
                for i in range(n_pairs)
            ]
            for i in range(n_pairs):
                # rank-1 mean-restore term c x ones seeds the accumulation
                nc.tensor.matmul(
                    out=accs[i][:, :],
                    lhsT=con_sb[0:1, c_cr : c_cr + 2 * H],
                    rhs=con_sb[0:1, c_ones : c_ones + n_slice],
                    start=True,
                    stop=False,
                    skip_group_check=True,
                )

            # ---- aggregation: aggT[h, m] += support_kb.T-stationary @ adjT ----
            # Every k-block issues 2*n_pairs accumulating matmuls; within a
            # pair the two matmuls target different PE column halves
            # (tile_position) and run concurrently on two m-slices.
            def agg_mm(a, t, j, i, u):
                kb = t * kb_per_tile + j
                m0 = (2 * i + u) * n_slice
                nc.tensor.matmul(
                    out=accs[i][u * H : (u + 1) * H, :],
                    lhsT=support_sb[:, kb * H : (kb + 1) * H],
                    rhs=a[:, j * rows + m0 : j * rows + m0 + n_slice],
                    start=False,
                    stop=(kb == nkb - 1),
                    tile_position=(0, u * H),
                    # the two column halves are disjoint partition groups in
                    # the same bank; the coarse zero-region group check can't
                    # express that
                    skip_group_check=True,
                )

            for t in range(n_tiles):
                a = apool.tile([KB, kb_per_tile * rows], qdt, tag="adj", name="a")
                nc.sync.dma_start(out=a[:], in_=adjT[t * KB : (t + 1) * KB, :])
                if t < n_tiles - 1:
                    for j in range(kb_per_tile):
                        for i in range(n_pairs):
                            for u in (0, 1):
                                agg_mm(a, t, j, i, u)
                else:
                    # last tile: finish pair 0 first so its epilogue overlaps
                    # pair 1's final matmuls
                    for i in range(n_pairs):
                        for j in range(kb_per_tile):
                            for u in (0, 1):
                                agg_mm(a, t, j, i, u)

            # ---- epilogue: relu(deg * aggT + b), in the scrambled layout ----
            o_sb = epool.tile([2 * H, hcols], f32, tag="o", name="o")
            for i in range(n_pairs):
                tmp = epool.tile([2 * H, n_slice], f32, tag="tmp", name="tmp")
                nc.vector.tensor_tensor(
                    out=tmp[:],
                    in0=accs[i][:],
                    in1=deg_sb[:, i * n_slice : (i + 1) * n_slice],
                    op=mybir.AluOpType.mult,
                )
                nc.scalar.activation(
                    out=o_sb[:, i * n_slice : (i + 1) * n_slice],
                    in_=tmp[:],
                    func=mybir.ActivationFunctionType.Relu,
                    bias=b_sb[:],
                )
                # per-pair output DMA overlaps the other pair's epilogue
                nc.scalar.dma_start(
                    out=outp[:, i * n_slice : (i + 1) * n_slice],
                    in_=o_sb[:, i * n_slice : (i + 1) * n_slice],
                )

    nc.compile()
    return nc


def pack_adjT(adjT_c, rows, kb_per_tile=KB_PER_TILE):
    """[n_nodes, rows] transposed adjacency shard -> DMA-tiled layout
    [n_tiles*128, kb_per_tile*rows]: row t*128+p concatenates the
    kb_per_tile k-block rows (kb_per_tile*t+j)*128+p, giving contiguous
    per-partition runs inside each DMA tile."""
    n_nodes = adjT_c.shape[0]
    n_tiles = n_nodes // (KB * kb_per_tile)
    return np.ascontiguousarray(
        adjT_c.reshape(n_tiles, kb_per_tile, KB, rows)
        .transpose(0, 2, 1, 3)
        .reshape(n_tiles * KB, kb_per_tile * rows)
    )


def pack_support(sup, dtype):
    """[n_nodes, H] support -> [128, nkb*H]: partition k, col kb*H+h holds
    support[kb*128 + k, h]."""
    n_nodes = sup.shape[0]
    nkb = n_nodes // KB
    return np.ascontiguousarray(
        sup.reshape(nkb, KB, H).transpose(1, 0, 2).reshape(KB, nkb * H).astype(dtype)
    )


def scramble_cols(v, rows):
    """[rows] vector -> [2, rows//2]: the two distinct rows of the kernel's
    scrambled layout (row u, col i*ns+n = v[i*2*ns + u*ns + n]); the kernel
    broadcasts row u to partitions u*64..u*64+63 via a K=2 selector matmul."""
    ns = min(512, rows // 2)
    npair = rows // (2 * ns)
    m = v.reshape(npair, 2, ns)  # [i, u, n]
    out = np.empty((2, npair * ns), dtype=v.dtype)
    for u in (0, 1):
        out[u, :] = m[:, u, :].reshape(npair * ns)
    return out


SEL = np.zeros((2, 2 * H), dtype=np.float32)
SEL[0, :H] = 1.0
SEL[1, H:] = 1.0


def unscramble_out(o, rows):
    """[128, rows//2] kernel output -> [rows, H] natural layout."""
    ns = min(512, rows // 2)
    npair = rows // (2 * ns)
    outT = np.empty((H, rows), dtype=o.dtype)
    for i in range(npair):
        for u in (0, 1):
            outT[:, (2 * i + u) * ns : (2 * i + u + 1) * ns] = o[
                u * H : (u + 1) * H, i * ns : (i + 1) * ns
            ]
    return outT.T


def make_in_maps(x, adj_matrix, degree_norm, W, b, adj_mode=ADJ_MODE,
                 kb_per_tile=KB_PER_TILE):
    """Shard the full inputs into per-core input maps (host-side, numpy)."""
    qdt = _ADJ_DT[adj_mode][1]
    sdt = np.float32 if adj_mode == "f32" else ml_dtypes.bfloat16
    center = adj_mode == "fp8"
    n_nodes = x.shape[0]
    rows = n_nodes // N_CORES
    ns = min(512, rows // 2)
    # support precomputed on host in fp32, shipped in sdt
    sup_f32 = x.astype(np.float32) @ np.asarray(W, np.float32)
    supm = pack_support(sup_f32, sdt)
    bf = np.ascontiguousarray(
        np.concatenate([np.asarray(b, np.float32)] * 2), dtype=np.float32
    ).reshape(2 * H, 1)
    if center:
        # exact rank-1 mean-restore: c = 0.5 * colsum(x @ W)
        c = 0.5 * sup_f32.astype(np.float64).sum(axis=0).astype(np.float32)
    else:
        c = np.zeros(H, np.float32)
    in_maps = []
    for ci in range(N_CORES):
        r0, r1 = ci * rows, (ci + 1) * rows
        adjT_c = adj_matrix[r0:r1, :].T.astype(np.float32)
        if center:
            adjT_c = adjT_c - np.float32(0.5)
        adjT_c = pack_adjT(
            np.ascontiguousarray(adjT_c.astype(qdt)), rows, kb_per_tile=kb_per_tile
        )
        deg_c = scramble_cols(
            np.ascontiguousarray(degree_norm[r0:r1].reshape(-1), np.float32), rows
        )
        # packed fp16 constants: [deg2 | sel | cr | ones] (see build_nc)
        hcols = rows // 2
        consts = np.zeros((2, hcols + 4 * H + ns), np.float16)
        consts[:, :hcols] = deg_c
        consts[:, hcols : hcols + 2 * H] = SEL
        consts[0, hcols + 2 * H : hcols + 4 * H] = np.concatenate([c, c])
        consts[0, hcols + 4 * H :] = 1.0
        in_maps.append({"adjT": adjT_c, "sup": supm, "b": bf, "consts": consts})
    return in_maps


_nc_cache = {}


def _get_nc():
    key = (ADJ_MODE, ADJ_BUFS, KB_PER_TILE, SUP_CHUNKS)
    if key not in _nc_cache:
        _nc_cache[key] = build_nc()
    return _nc_cache[key]


def kernel(x, adj_matrix, degree_norm, W, b):
    x = np.asarray(x)
    adj_matrix = np.asarray(adj_matrix)
    degree_norm = np.asarray(degree_norm)
    W = np.asarray(W)
    b = np.asarray(b)

    nc = _get_nc()
    in_maps = make_in_maps(x, adj_matrix, degree_norm, W, b)
    try:
        res = run_bass_kernel_spmd(nc, in_maps, core_ids=list(range(N_CORES)))
    except Exception:
        # transient NRT_EXEC_UNIT_UNRECOVERABLE after an aborted prior run
        # heals after touching the devices once; retry a single time
        try:
            import jax, jax.numpy as jnp  # noqa: E401

            for d in jax.devices():
                jnp.add(jax.device_put(jnp.ones((2, 2)), d), 1.0).block_until_ready()
        except Exception:
            pass
        res = run_bass_kernel_spmd(nc, in_maps, core_ids=list(range(N_CORES)))
    out = np.empty((N_NODES, H), dtype=np.float32)
    for c in range(N_CORES):
        out[c * ROWS : (c + 1) * ROWS, :] = unscramble_out(res.results[c]["out"], ROWS)
    return out
